# revision 5
# baseline (speedup 1.0000x reference)
"""Trainium2 Bass kernel for DepthwiseXCorr (SiamRPN-style) model.

Pipeline (per sample): conv3x3+BN+ReLU on kernel & search branches,
depthwise cross-correlation, 1x1 conv + BN + ReLU head, 1x1 conv + bias.

Sharding: data-parallel over batch across 8 NeuronCores (8 samples each),
weights replicated.  BN is folded into conv weights on the host.

Layout on device: channels on SBUF partitions (2 chunks of 128), spatial x
batch on the free dimension.  Convolutions run as 9 shifted matmuls in bf16.
The depthwise xcorr (16 (sample, channel-chunk) pairs of 25 taps each) is
spread across all four compute engines, per-pair lane assignment:
  T: TensorE accumulates diag(k_tap) @ shifted-window matmuls in PSUM;
     the 25 diagonals per pair are built by DVE tensor_scalar from a bf16
     identity (or by ACT scale-copy).
  V: VectorE scalar_tensor_tensor chain over strided 25x25 windows.
  M: products (win * k_tap -> bf16, via DVE tensor_scalar or ACT
     scale-copy) summed by a DVE pairwise fp16 tree.
  G: products as in M; DVE adds pairs of products (level 1), then the
     otherwise-idle GpSimd engine chain-adds the 12 partials in f32.
Head matmuls read packed-625 feature tiles (fp16 or f32r depending on the
lane) with matching-dtype h1 weight copies.  Head emission is delayed a few
samples so the slow GpSimd chains never stall the TensorE queue.
"""

import sys

if "/opt/trn_rl_repo" not in sys.path:
    sys.path.insert(0, "/opt/trn_rl_repo")

from contextlib import ExitStack

import ml_dtypes
import numpy as np

import concourse.bass as bass
import concourse.tile as tile
from concourse import bacc, mybir
from concourse import library_config
from concourse.bass_utils import run_bass_kernel_spmd

EPS = 1e-5
NCORES = 8
B, C, HID, OUT = 64, 256, 256, 10
BPC = B // NCORES  # samples per core
P = 128
KC = C // P  # channel chunks (2)
F32 = mybir.dt.float32
F32R = mybir.dt.float32r
F16 = mybir.dt.float16
BF16 = mybir.dt.bfloat16
AF = mybir.ActivationFunctionType
OP = mybir.AluOpType

# ---- xcorr lane assignment, index p = b*2 + cc ----
# ("T", diag_eng): PE diag-matmuls; diag_eng "dve" | "act"
# ("V",):          DVE STT chain
# ("M", prod_eng): products + DVE fp16 tree; prod_eng "dve" | "act"
# ("G", prod_eng, l1): products + (l1: DVE level-1 then GS chains 12
#                  partials | not l1: GS chains all 25 products)
XC_CFG = [
    ("G", "act", True),   # 0  (b0, cc0)
    ("T", "dve"),         # 1  (b0, cc1)
    ("G", "act", True),   # 2  (b1, cc0)
    ("V",),               # 3  (b1, cc1)
    ("G", "act", True),   # 4  (b2, cc0)
    ("T", "dve"),         # 5  (b2, cc1)
    ("G", "act", True),   # 6  (b3, cc0)
    ("V",),               # 7  (b3, cc1)
    ("G", "dve", True),   # 8  (b4, cc0)
    ("T", "dve"),         # 9  (b4, cc1)
    ("G", "dve", True),   # 10 (b5, cc0)
    ("V",),               # 11 (b5, cc1)
    ("T", "dve"),         # 12 (b6, cc0)
    ("M", "act"),         # 13 (b6, cc1)
    ("G", "act", True),   # 14 (b7, cc0)
    ("M", "act"),         # 15 (b7, cc1)
]
HEAD_DELAY = 3  # emit head(b - HEAD_DELAY) after xcorr(b)

LAST_RESULTS = None  # BassKernelResults of the most recent run (for profiling)

_prog_cache = {}


def _emit(nc, tc, ctx, d):
    """Emit the per-core program.  d maps dram tensor name -> handle."""
    wp = ctx.enter_context(tc.tile_pool(name="weights", bufs=1))
    srp = ctx.enter_context(tc.tile_pool(name="srelu", bufs=1))
    krp = ctx.enter_context(tc.tile_pool(name="krelu", bufs=1))
    kp = ctx.enter_context(tc.tile_pool(name="kern", bufs=1))
    sp = ctx.enter_context(tc.tile_pool(name="search", bufs=8))
    featfp = ctx.enter_context(tc.tile_pool(name="featf", bufs=6))
    feathp = ctx.enter_context(tc.tile_pool(name="feath", bufs=6))
    diagp = ctx.enter_context(tc.tile_pool(name="diag", bufs=2))
    prodp = ctx.enter_context(tc.tile_pool(name="prod", bufs=16))
    partp = ctx.enter_context(tc.tile_pool(name="part", bufs=14))
    xrp = ctx.enter_context(tc.tile_pool(name="xrelu", bufs=4))
    outp = ctx.enter_context(tc.tile_pool(name="outs", bufs=1))
    ps_conv = ctx.enter_context(tc.tile_pool(name="ps_conv", bufs=3, space="PSUM"))
    ps_x = ctx.enter_context(tc.tile_pool(name="ps_x", bufs=3, space="PSUM"))
    ps_hd = ctx.enter_context(tc.tile_pool(name="ps_hd", bufs=2, space="PSUM"))

    # ---- weights / constants into SBUF ----
    s0_sb = []
    for kc in range(KC):
        t = sp.tile([P, 31, 32], BF16, tag="sin", name=f"sin{kc}_0")
        nc.sync.dma_start(t[:], d["s_in"].ap()[kc, :, 0])
        s0_sb.append(t)
    csw_sb, ckw_sb, h1w_sb, h1w16_sb, h2w_sb = [], [], [], [], []
    csb_sb, ckb_sb, h1b_sb = [], [], []
    for kc in range(KC):
        t = wp.tile([P, 9 * 2 * P], BF16, tag=f"csw{kc}")
        nc.sync.dma_start(t[:], d["csw"].ap()[kc])
        csw_sb.append(t)
    k_sb = []
    for kc in range(KC):
        t = kp.tile([P, BPC, 9, 9], BF16, tag=f"kin{kc}")
        nc.sync.dma_start(t[:], d["k_in"].ap()[kc])
        k_sb.append(t)
    for kc in range(KC):
        t = wp.tile([P, 9 * 2 * P], BF16, tag=f"ckw{kc}")
        nc.sync.dma_start(t[:], d["ckw"].ap()[kc])
        ckw_sb.append(t)
    for kc in range(KC):
        t = wp.tile([P, 2 * P], F32R, tag=f"h1w{kc}")
        nc.sync.dma_start(t[:], d["h1w"].ap()[kc])
        h1w_sb.append(t)
        t = wp.tile([P, 2 * P], F16, tag=f"h1w16{kc}")
        nc.sync.dma_start(t[:], d["h1w16"].ap()[kc])
        h1w16_sb.append(t)
        t = wp.tile([P, OUT], F32R, tag=f"h2w{kc}")
        nc.sync.dma_start(t[:], d["h2w"].ap()[kc])
        h2w_sb.append(t)
    for mc in range(KC):
        t = wp.tile([P, 1], F32, tag=f"csb{mc}")
        nc.sync.dma_start(t[:], d["cs_bias"].ap()[mc])
        csb_sb.append(t)
        t = wp.tile([P, 1], F32, tag=f"ckb{mc}")
        nc.sync.dma_start(t[:], d["ck_bias"].ap()[mc])
        ckb_sb.append(t)
        t = wp.tile([P, 1], F32, tag=f"h1b{mc}")
        nc.sync.dma_start(t[:], d["h1_bias"].ap()[mc])
        h1b_sb.append(t)
    h2b_sb = wp.tile([OUT, 1], F32, tag="h2b")
    nc.sync.dma_start(h2b_sb[:], d["h2_bias"].ap())
    ident_sb = wp.tile([P, P], BF16, tag="ident")
    nc.sync.dma_start(ident_sb[:], d["ident"].ap())

    nc.gpsimd.load_library(library_config.standard)

    # ---- inputs + convolutions (rotating per-sample search tiles) ----
    krelu_sb = [krp.tile([P, BPC * 25], F32, tag=f"krelu{mc}", name=f"krelu{mc}") for mc in range(KC)]
    srelu_sb = [srp.tile([P, BPC, 29, 30], BF16, tag=f"srelu{mc}", name=f"srelu{mc}") for mc in range(KC)]
    for mc in range(KC):
        nc.vector.memset(srelu_sb[mc][:, :, :, 29:30], 0.0)

    out_sb = outp.tile([OUT, BPC * 625], F32, tag="osb")

    def kscalar(cc, b, tap):
        return krelu_sb[cc][:, b * 25 + tap:b * 25 + tap + 1]

    def win(cc, b, tap, r0=0, nr=25):
        dy, dx = tap // 5, tap % 5
        return srelu_sb[cc][:, b, dy + r0:dy + r0 + nr, dx:dx + 25]

    # feat tiles per pair index, plus their dtype flavor for head1
    feat_tiles = {}

    def emit_conv_search(b, s_sb):
        for mc in range(KC):
            for y0, nr in ((0, 15), (15, 14)):
                pss = ps_conv.tile([P, nr, 29], F32, tag="pss")
                i = 0
                for tap in range(9):
                    dy, dx = tap // 3, tap % 3
                    for kc in range(KC):
                        lhsT = csw_sb[kc][:, tap * 2 * P + mc * P:tap * 2 * P + (mc + 1) * P]
                        rhs = s_sb[kc][:, y0 + dy:y0 + dy + nr, dx:dx + 29]
                        nc.tensor.matmul(pss[:], lhsT, rhs,
                                         start=(i == 0), stop=(i == 17))
                        i += 1
                nc.scalar.activation(srelu_sb[mc][:, b, y0:y0 + nr, 0:29], pss[:],
                                     AF.Relu, bias=csb_sb[mc][:])

    def emit_conv_kernel():
        for mc in range(KC):
            psk = ps_conv.tile([P, BPC, 6, 6], F32, tag="pss")
            i = 0
            for tap in range(9):
                dy, dx = tap // 3, tap % 3
                for kc in range(KC):
                    lhsT = ckw_sb[kc][:, tap * 2 * P + mc * P:tap * 2 * P + (mc + 1) * P]
                    rhs = k_sb[kc][:, :, dy:dy + 6, dx:dx + 6]
                    nc.tensor.matmul(psk[:], lhsT, rhs, start=(i == 0), stop=(i == 17))
                    i += 1
            nc.scalar.activation(krelu_sb[mc][:], psk[:, :, 0:5, 0:5], AF.Relu,
                                 bias=ckb_sb[mc][:])

    def emit_products(cc, b, prod_eng):
        prods = []
        for tap in range(25):
            pr = prodp.tile([P, 625], BF16, tag="prod")
            if prod_eng == "dve":
                nc.vector.tensor_scalar(pr[:], win(cc, b, tap),
                                        kscalar(cc, b, tap), None, OP.mult)
            else:
                nc.scalar.activation(pr[:].rearrange("p (y x) -> p y x", x=25),
                                     win(cc, b, tap), AF.Copy,
                                     scale=kscalar(cc, b, tap))
            prods.append(pr)
        return prods

    def emit_pair(pidx):
        b, cc = pidx // 2, pidx % 2
        cfg = XC_CFG[pidx]
        lane = cfg[0]
        if lane == "T":
            dg = diagp.tile([P, 25, P], BF16, tag="diag")
            for tap in range(25):
                if cfg[1] == "dve":
                    nc.vector.tensor_scalar(dg[:, tap, :], ident_sb[:],
                                            kscalar(cc, b, tap), None, OP.mult)
                else:
                    nc.scalar.activation(dg[:, tap, :], ident_sb[:], AF.Copy,
                                         scale=kscalar(cc, b, tap))
            ft = feathp.tile([P, 626], F16, tag="feath")
            nc.vector.memset(ft[:, 625:626], 0.0)
            ps_a = ps_x.tile([P, 325], F32, tag="psx")
            ps_b = ps_x.tile([P, 300], F32, tag="psx")
            for tap in range(25):
                nc.tensor.matmul(ps_a[:].rearrange("p (y x) -> p y x", x=25),
                                 dg[:, tap, :], win(cc, b, tap, 0, 13),
                                 start=(tap == 0), stop=(tap == 24))
                nc.tensor.matmul(ps_b[:].rearrange("p (y x) -> p y x", x=25),
                                 dg[:, tap, :], win(cc, b, tap, 13, 12),
                                 start=(tap == 0), stop=(tap == 24))
            nc.scalar.activation(ft[:, 0:325], ps_a[:], AF.Copy)
            nc.scalar.activation(ft[:, 325:625], ps_b[:], AF.Copy)
            feat_tiles[pidx] = (ft, F16)
        elif lane == "V":
            ft = featfp.tile([P, 626], F32R, tag="featf")
            nc.vector.memset(ft[:, 625:626].bitcast(F32), 0.0)
            fv = ft[:, 0:625].rearrange("p (y x) -> p y x", x=25)
            nc.vector.tensor_scalar(fv, win(cc, b, 0),
                                    kscalar(cc, b, 0), None, OP.mult)
            for tap in range(1, 25):
                nc.vector.scalar_tensor_tensor(fv, win(cc, b, tap),
                                               kscalar(cc, b, tap),
                                               fv.bitcast(F32), OP.mult, OP.add)
            feat_tiles[pidx] = (ft, F32R)
        elif lane == "M":
            prods = emit_products(cc, b, cfg[1])
            items = list(prods)
            while len(items) > 1:
                nxt = []
                for i in range(0, len(items) - 1, 2):
                    if len(items) == 2:
                        res = feathp.tile([P, 626], F16, tag="feath")
                        nc.vector.memset(res[:, 625:626], 0.0)
                    else:
                        res = partp.tile([P, 625], F16, tag="part")
                    nc.vector.tensor_tensor(res[:, 0:625], items[i][:],
                                            items[i + 1][:], OP.add)
                    nxt.append(res)
                if len(items) % 2:
                    nxt.append(items[-1])
                items = nxt
            feat_tiles[pidx] = (items[0], F16)
        else:  # "G"
            prods = emit_products(cc, b, cfg[1])
            ft = featfp.tile([P, 626], F32R, tag="featf")
            nc.vector.memset(ft[:, 625:626].bitcast(F32), 0.0)
            acc = partp.tile([P, 625], F32, tag="gacc")
            if cfg[2]:  # DVE level-1, then GS chains 12 partials + leftover
                parts = []
                for i in range(0, 24, 2):
                    pt = partp.tile([P, 625], F16, tag="part")
                    nc.vector.tensor_tensor(pt[:], prods[i][:],
                                            prods[i + 1][:], OP.add)
                    parts.append(pt)
                nc.gpsimd.tensor_tensor(acc[:], parts[0][:], parts[1][:], OP.add)
                for pt in parts[2:]:
                    nc.gpsimd.tensor_tensor(acc[:], pt[:], acc[:].bitcast(F32), OP.add)
                nc.vector.tensor_tensor(ft[:, 0:625], prods[24][:], acc[:].bitcast(F32), OP.add)
            else:  # GS chains all 25 products
                nc.gpsimd.tensor_tensor(acc[:], prods[0][:], prods[1][:], OP.add)
                for pr in prods[2:24]:
                    nc.gpsimd.tensor_tensor(acc[:], pr[:], acc[:].bitcast(F32), OP.add)
                nc.vector.tensor_tensor(ft[:, 0:625], prods[24][:], acc[:].bitcast(F32), OP.add)
            feat_tiles[pidx] = (ft, F32R)

    def emit_head(b):
        feat = [feat_tiles.pop(b * 2 + cc) for cc in range(KC)]
        xr = []
        for mc in range(KC):
            t = xrp.tile([P, 626], F32R, tag="xr", name=f"xr{b}_{mc}")
            nc.vector.memset(t[:, 625:626].bitcast(F32), 0.0)
            xr.append(t)
        for mc in range(KC):
            for o0, n, nv in ((0, 320, 320), (320, 306, 305)):
                ph = ps_hd.tile([P, n], F32, tag="pshd")
                for kc in range(KC):
                    ft, fdt = feat[kc]
                    lhsT = (h1w16_sb if fdt == F16 else h1w_sb)[kc][:, mc * P:(mc + 1) * P]
                    nc.tensor.matmul(ph[:], lhsT, ft[:, o0:o0 + n],
                                     start=(kc == 0), stop=(kc == 1))
                nc.scalar.activation(xr[mc][:, o0:o0 + nv], ph[:, 0:nv],
                                     AF.Relu, bias=h1b_sb[mc][:])
        for o0, n, nv in ((0, 320, 320), (320, 306, 305)):
            po = ps_hd.tile([OUT, n], F32, tag="pshd")
            for kc in range(KC):
                nc.tensor.matmul(po[:], h2w_sb[kc][:],
                                 xr[kc][:, o0:o0 + n],
                                 start=(kc == 0), stop=(kc == 1))
            nc.scalar.activation(out_sb[:, b * 625 + o0:b * 625 + o0 + nv], po[:, 0:nv],
                                 AF.Identity, bias=h2b_sb[:])

    # ---- main pipeline ----
    for b in range(BPC):
        if b == 0:
            s_sb = s0_sb
        else:
            s_sb = []
            for kc in range(KC):
                t = sp.tile([P, 31, 32], BF16, tag="sin", name=f"sin{kc}_{b}")
                nc.sync.dma_start(t[:], d["s_in"].ap()[kc, :, b])
                s_sb.append(t)
        emit_conv_search(b, s_sb)
        if b == 0:
            emit_conv_kernel()
        for cc in range(KC):
            emit_pair(b * 2 + cc)
        if b >= HEAD_DELAY:
            emit_head(b - HEAD_DELAY)
    for b in range(BPC - HEAD_DELAY, BPC):
        emit_head(b)

    nc.sync.dma_start(d["out"].ap(), out_sb[:])


def _build_program():
    if "nc" in _prog_cache:
        return _prog_cache["nc"]
    nc = bacc.Bacc("TRN2", target_bir_lowering=False, debug=False,
                   num_devices=NCORES)
    d = {}
    d["s_in"] = nc.dram_tensor("s_in", [KC, P, BPC, 31, 32], BF16, kind="ExternalInput")
    d["k_in"] = nc.dram_tensor("k_in", [KC, P, BPC, 9, 9], BF16, kind="ExternalInput")
    d["csw"] = nc.dram_tensor("csw", [KC, P, 9, 2, P], BF16, kind="ExternalInput")
    d["ckw"] = nc.dram_tensor("ckw", [KC, P, 9, 2, P], BF16, kind="ExternalInput")
    d["cs_bias"] = nc.dram_tensor("cs_bias", [KC, P, 1], F32, kind="ExternalInput")
    d["ck_bias"] = nc.dram_tensor("ck_bias", [KC, P, 1], F32, kind="ExternalInput")
    d["h1w"] = nc.dram_tensor("h1w", [KC, P, 2, P], F32R, kind="ExternalInput")
    d["h1w16"] = nc.dram_tensor("h1w16", [KC, P, 2, P], F16, kind="ExternalInput")
    d["h1_bias"] = nc.dram_tensor("h1_bias", [KC, P, 1], F32, kind="ExternalInput")
    d["h2w"] = nc.dram_tensor("h2w", [KC, P, OUT], F32R, kind="ExternalInput")
    d["h2_bias"] = nc.dram_tensor("h2_bias", [OUT, 1], F32, kind="ExternalInput")
    d["ident"] = nc.dram_tensor("ident", [P, P], BF16, kind="ExternalInput")
    d["out"] = nc.dram_tensor("out", [OUT, BPC * 625], F32, kind="ExternalOutput")

    with tile.TileContext(nc) as tc:
        with ExitStack() as ctx:
            _emit(nc, tc, ctx, d)
    nc.compile()
    _prog_cache["nc"] = nc
    return nc


def kernel(**inputs):
    global LAST_RESULTS
    f32 = lambda x: np.ascontiguousarray(np.asarray(x), dtype=np.float32)
    kern, search = f32(inputs["kernel"]), f32(inputs["search"])

    # fold BN into conv weights / biases
    cks = f32(inputs["ck_g"]) / np.sqrt(f32(inputs["ck_v"]) + EPS)
    ckw_f = f32(inputs["ck_w"]) * cks[:, None, None, None]
    ckb = f32(inputs["ck_b"]) - f32(inputs["ck_m"]) * cks
    css = f32(inputs["cs_g"]) / np.sqrt(f32(inputs["cs_v"]) + EPS)
    csw_f = f32(inputs["cs_w"]) * css[:, None, None, None]
    csb = f32(inputs["cs_b"]) - f32(inputs["cs_m"]) * css
    h1s = f32(inputs["h_g"]) / np.sqrt(f32(inputs["h_v"]) + EPS)
    h1w_f = f32(inputs["h1_w"]) * h1s[:, None]
    h1b = f32(inputs["h_b"]) - f32(inputs["h_m"]) * h1s

    h1w_t = np.ascontiguousarray(h1w_f.transpose(1, 0).reshape(KC, P, 2, P))
    shared = {
        "csw": np.ascontiguousarray(
            csw_f.transpose(1, 2, 3, 0).reshape(KC, P, 9, 2, P)).astype(ml_dtypes.bfloat16),
        "ckw": np.ascontiguousarray(
            ckw_f.transpose(1, 2, 3, 0).reshape(KC, P, 9, 2, P)).astype(ml_dtypes.bfloat16),
        "cs_bias": csb.reshape(KC, P, 1),
        "ck_bias": ckb.reshape(KC, P, 1),
        "h1w": h1w_t,
        "h1w16": h1w_t.astype(np.float16),
        "h1_bias": h1b.reshape(KC, P, 1),
        "h2w": np.ascontiguousarray(f32(inputs["h2_w"]).transpose(1, 0).reshape(KC, P, OUT)),
        "h2_bias": f32(inputs["h2_b"]).reshape(OUT, 1),
        "ident": np.eye(P, dtype=ml_dtypes.bfloat16),
    }
    in_maps = []
    for i in range(NCORES):
        sl = slice(i * BPC, (i + 1) * BPC)
        m = dict(shared)
        s_pad = np.zeros((KC, P, BPC, 31, 32), ml_dtypes.bfloat16)
        s_pad[..., :31] = search[sl].transpose(1, 0, 2, 3).reshape(KC, P, BPC, 31, 31)
        m["s_in"] = s_pad
        k_pad = np.zeros((KC, P, BPC, 9, 9), ml_dtypes.bfloat16)
        k_pad[..., :7, :7] = kern[sl].transpose(1, 0, 2, 3).reshape(KC, P, BPC, 7, 7)
        m["k_in"] = k_pad
        in_maps.append(m)

    nc = _build_program()
    res = run_bass_kernel_spmd(nc, in_maps, core_ids=list(range(NCORES)))
    LAST_RESULTS = res
    out = np.empty((B, OUT, 25, 25), dtype=np.float32)
    for i in range(NCORES):
        o = res.results[i]["out"].reshape(OUT, BPC, 25, 25)
        out[i * BPC:(i + 1) * BPC] = o.transpose(1, 0, 2, 3)
    return out


# revision 12
# speedup vs baseline: 1.0372x; 1.0372x over previous
"""Trainium2 Bass kernel for DepthwiseXCorr (SiamRPN-style) model.

Pipeline (per sample): conv3x3+BN+ReLU on kernel & search branches,
depthwise cross-correlation, 1x1 conv + BN + ReLU head, 1x1 conv + bias.

Sharding: data-parallel over batch across 8 NeuronCores (8 samples each),
weights replicated.  BN is folded into conv weights on the host.

Layout on device: channels on SBUF partitions (2 chunks of 128), spatial x
batch on the free dimension.  Convolutions run as 9 shifted matmuls in
bf16, activations flow in fp16.  The depthwise xcorr (16 (sample,
channel-chunk) pairs of 25 taps each) is spread across all four compute
engines via per-pair lanes:
  T:   PE accumulates diag(k_tap) @ shifted-window matmuls in PSUM; the
       diagonals are built by DVE tensor_scalar from an fp16 identity.
  V:   DVE scalar_tensor_tensor chain over strided 25x25 windows.
  Md:  DVE tensor_scalar products (fp16) + DVE pairwise fp16 tree.
  Gal: ACT scale-copy products; DVE adds product pairs (level 1); the
       otherwise-idle GpSimd chain-adds the 12 partials in f32.
  Gdf: DVE products; GpSimd chain-adds all of them.
Because every engine queue executes in emission order, the whole program
is list-scheduled at build time: each instruction is an op-graph node with
a measured cost estimate, and a greedy earliest-start scheduler decides
the global emission order so no queue convoys behind slow work.
"""

import sys

if "/opt/trn_rl_repo" not in sys.path:
    sys.path.insert(0, "/opt/trn_rl_repo")

import heapq
from contextlib import ExitStack

import ml_dtypes
import numpy as np

import concourse.bass as bass
import concourse.tile as tile
from concourse import bacc, mybir
from concourse import library_config
from concourse.bass_utils import run_bass_kernel_spmd

EPS = 1e-5
NCORES = 8
B, C, HID, OUT = 64, 256, 256, 10
BPC = B // NCORES  # samples per core
P = 128
KC = C // P  # channel chunks (2)
F32 = mybir.dt.float32
F32R = mybir.dt.float32r
F16 = mybir.dt.float16
BF16 = mybir.dt.bfloat16
AF = mybir.ActivationFunctionType
OP = mybir.AluOpType

# xcorr lane per pair index p = b*2 + cc:
#   "T" PE diag-mm | "V" DVE STT | "Md" DVE prods+tree |
#   "Gal" ACT prods + DVE L1 + GS chain | "Gdf" DVE prods + GS full chain
XC_CFG = [
    "Gal", "Gdf",   # b0
    "Gal", "T",     # b1
    "Gal", "Gdf",   # b2
    "Gal", "T",     # b3
    "Gal", "V",     # b4
    "Gal", "T",     # b5
    "V",   "T",     # b6
    "V",   "T",     # b7
]

LAST_RESULTS = None  # BassKernelResults of the most recent run (for profiling)

_prog_cache = {}


class _Op:
    __slots__ = ("eng", "cost", "deps", "fn", "idx", "succ", "t_ready", "t_end")

    def __init__(self, eng, cost, deps, fn, idx):
        self.eng, self.cost, self.deps, self.fn, self.idx = eng, cost, deps, fn, idx
        self.succ = []


class _Sched:
    """Op-graph builder + greedy earliest-start list scheduler."""

    def __init__(self):
        self.ops = []

    def op(self, eng, cost, deps, fn):
        o = _Op(eng, cost, [d for d in deps if d is not None], fn, len(self.ops))
        self.ops.append(o)
        return o

    def order(self):
        indeg = [0] * len(self.ops)
        for o in self.ops:
            for d in o.deps:
                d.succ.append(o)
            indeg[o.idx] = len(o.deps)
        clock = {}
        ready = []
        for o in self.ops:
            if indeg[o.idx] == 0:
                o.t_ready = 0.0
                heapq.heappush(ready, (0.0, o.idx))
        out = []
        while ready:
            cand = []
            while ready and len(cand) < 24:
                cand.append(heapq.heappop(ready))
            best, bi = None, -1
            for i, (tr, idx) in enumerate(cand):
                o = self.ops[idx]
                st = max(clock.get(o.eng, 0.0), tr)
                key = (st, tr, idx)
                if best is None or key < best:
                    best, bi = key, i
            tr, idx = cand.pop(bi)
            for c in cand:
                heapq.heappush(ready, c)
            o = self.ops[idx]
            st = max(clock.get(o.eng, 0.0), tr)
            o.t_end = st + o.cost
            clock[o.eng] = o.t_end
            out.append(o)
            for s in o.succ:
                indeg[s.idx] -= 1
                if indeg[s.idx] == 0:
                    s.t_ready = max((d.t_end for d in s.deps), default=0.0)
                    heapq.heappush(ready, (s.t_ready, s.idx))
        assert len(out) == len(self.ops), f"{len(out)} vs {len(self.ops)}"
        return out


class _PoolGate:
    """Approximates a tile pool's bufs-deep ring for the scheduler: the
    k-th allocation waits for the consumers of allocation k-bufs.
    alloc() returns (slot_id, deps); register consumers with use(slot, op)."""

    def __init__(self, bufs):
        self.bufs = bufs
        self.hist = []
        self.last_alloc = None

    def alloc(self):
        k = len(self.hist)
        self.hist.append([])
        deps = list(self.hist[k - self.bufs]) if k >= self.bufs else []
        if self.last_alloc is not None:
            deps.append(self.last_alloc)
        return k, deps

    def chain(self, o):
        """Register o as the op that performs this allocation (ring order)."""
        self.last_alloc = o
        return o

    def use(self, slot, o):
        if o is not None:
            self.hist[slot].append(o)
        return o


def _emit(nc, tc, ctx, d):
    """Build the op graph for the per-core program, schedule it, emit."""
    wp = ctx.enter_context(tc.tile_pool(name="weights", bufs=1))
    srp = ctx.enter_context(tc.tile_pool(name="srelu", bufs=1))
    krp = ctx.enter_context(tc.tile_pool(name="krelu", bufs=1))
    kp = ctx.enter_context(tc.tile_pool(name="kern", bufs=1))
    sp = ctx.enter_context(tc.tile_pool(name="search", bufs=8))
    featp = ctx.enter_context(tc.tile_pool(name="feat", bufs=6))
    diagp = ctx.enter_context(tc.tile_pool(name="diag", bufs=2))
    prodp = ctx.enter_context(tc.tile_pool(name="prod", bufs=16))
    partp = ctx.enter_context(tc.tile_pool(name="part", bufs=12))
    gaccp = ctx.enter_context(tc.tile_pool(name="gacc", bufs=3))
    xrp = ctx.enter_context(tc.tile_pool(name="xrelu", bufs=4))
    outp = ctx.enter_context(tc.tile_pool(name="outs", bufs=1))
    ps_conv = ctx.enter_context(tc.tile_pool(name="ps_conv", bufs=3, space="PSUM"))
    ps_x = ctx.enter_context(tc.tile_pool(name="ps_x", bufs=3, space="PSUM"))
    ps_hd = ctx.enter_context(tc.tile_pool(name="ps_hd", bufs=2, space="PSUM"))

    # ---- preamble: weights / constants into SBUF (unscheduled) ----
    csw_sb, ckw_sb, h1w_sb, h1w16_sb, h2w_sb = [], [], [], [], []
    csb_sb, ckb_sb, h1b_sb = [], [], []
    for kc in range(KC):
        t = wp.tile([P, 9 * 2 * P], BF16, tag=f"csw{kc}")
        nc.sync.dma_start(t[:], d["csw"].ap()[kc])
        csw_sb.append(t)
    k_sb = []
    for kc in range(KC):
        t = kp.tile([P, BPC, 9, 9], BF16, tag=f"kin{kc}")
        nc.sync.dma_start(t[:], d["k_in"].ap()[kc])
        k_sb.append(t)
    for kc in range(KC):
        t = wp.tile([P, 9 * 2 * P], BF16, tag=f"ckw{kc}")
        nc.sync.dma_start(t[:], d["ckw"].ap()[kc])
        ckw_sb.append(t)
    for kc in range(KC):
        t = wp.tile([P, 2 * P], F32R, tag=f"h1w{kc}")
        nc.sync.dma_start(t[:], d["h1w"].ap()[kc])
        h1w_sb.append(t)
        t = wp.tile([P, 2 * P], F16, tag=f"h1w16{kc}")
        nc.sync.dma_start(t[:], d["h1w16"].ap()[kc])
        h1w16_sb.append(t)
        t = wp.tile([P, OUT], F32R, tag=f"h2w{kc}")
        nc.sync.dma_start(t[:], d["h2w"].ap()[kc])
        h2w_sb.append(t)
    for mc in range(KC):
        t = wp.tile([P, 1], F32, tag=f"csb{mc}")
        nc.sync.dma_start(t[:], d["cs_bias"].ap()[mc])
        csb_sb.append(t)
        t = wp.tile([P, 1], F32, tag=f"ckb{mc}")
        nc.sync.dma_start(t[:], d["ck_bias"].ap()[mc])
        ckb_sb.append(t)
        t = wp.tile([P, 1], F32, tag=f"h1b{mc}")
        nc.sync.dma_start(t[:], d["h1_bias"].ap()[mc])
        h1b_sb.append(t)
    h2b_sb = wp.tile([OUT, 1], F32, tag="h2b")
    nc.sync.dma_start(h2b_sb[:], d["h2_bias"].ap())
    ident_sb = wp.tile([P, P], F16, tag="ident")
    nc.sync.dma_start(ident_sb[:], d["ident"].ap())

    nc.gpsimd.load_library(library_config.standard)

    krelu_sb = [krp.tile([P, BPC * 25], F32, tag=f"krelu{mc}", name=f"krelu{mc}") for mc in range(KC)]
    srelu_sb = [srp.tile([P, BPC, 29, 30], F16, tag=f"srelu{mc}", name=f"srelu{mc}") for mc in range(KC)]
    for mc in range(KC):
        nc.vector.memset(srelu_sb[mc][:, :, :, 29:30], 0.0)
    out_sb = outp.tile([OUT, BPC * 625], F32, tag="osb")

    def kscalar(cc, b, tap):
        return krelu_sb[cc][:, b * 25 + tap:b * 25 + tap + 1]

    def win(cc, b, tap, r0=0, nr=25):
        dy, dx = tap // 5, tap % 5
        return srelu_sb[cc][:, b, dy + r0:dy + r0 + nr, dx:dx + 25]

    # ---- scheduled section ----
    S = _Sched()
    g_psc = _PoolGate(2)
    g_ff = _PoolGate(5)
    g_fh = _PoolGate(5)
    g_xr = _PoolGate(3)
    g_psx = _PoolGate(1)   # 2 banks per T pair, pool has 3 tiles -> 1.5; be safe
    g_pshd = _PoolGate(1)
    g_prod = _PoolGate(14)
    g_part = _PoolGate(10)
    g_gacc = _PoolGate(3)
    g_diag = _PoolGate(2)

    tiles = {}   # runtime tile registry, filled by closures
    s_tiles = {}
    dma_ops = {}
    for b in range(BPC):
        def dma_fn(b=b):
            ts = []
            for kc in range(KC):
                t = sp.tile([P, 31, 32], BF16, tag="sin", name=f"sin{kc}_{b}")
                nc.sync.dma_start(t[:], d["s_in"].ap()[kc, :, b])
                ts.append(t)
            s_tiles[b] = ts
        dma_ops[b] = S.op("DMA", 1500, [], dma_fn)

    srelu_ready = {(b, mc): [] for b in range(BPC) for mc in range(KC)}
    krelu_ready = []

    def build_conv(b):
        for mc in range(KC):
            for y0, nr in ((0, 15), (15, 14)):
                slot, gate = g_psc.alloc()
                prev = None
                for tap in range(9):
                    dy, dx = tap // 3, tap % 3
                    for kc in range(KC):
                        i = tap * 2 + kc

                        def mm_fn(b=b, mc=mc, y0=y0, nr=nr, tap=tap, dy=dy, dx=dx, kc=kc, i=i):
                            if i == 0:
                                tiles[("pss", b, mc, y0)] = ps_conv.tile([P, nr, 29], F32, tag="pss")
                            pss = tiles[("pss", b, mc, y0)]
                            lhsT = csw_sb[kc][:, tap * 2 * P + mc * P:tap * 2 * P + (mc + 1) * P]
                            rhs = s_tiles[b][kc][:, y0 + dy:y0 + dy + nr, dx:dx + 29]
                            nc.tensor.matmul(pss[:], lhsT, rhs,
                                             start=(i == 0), stop=(i == 17))
                        deps = (gate + [dma_ops[b]]) if i == 0 else [prev]
                        prev = S.op("PE", 200, deps, mm_fn)
                        if i == 0:
                            g_psc.chain(prev)

                def relu_fn(b=b, mc=mc, y0=y0, nr=nr):
                    nc.scalar.activation(srelu_sb[mc][:, b, y0:y0 + nr, 0:29],
                                         tiles[("pss", b, mc, y0)][:],
                                         AF.Relu, bias=csb_sb[mc][:])
                r = S.op("ACT", 560, [prev], relu_fn)
                g_psc.use(slot, r)
                srelu_ready[(b, mc)].append(r)

    def build_conv_kernel():
        for mc in range(KC):
            slot, gate = g_psc.alloc()
            prev = None
            for tap in range(9):
                dy, dx = tap // 3, tap % 3
                for kc in range(KC):
                    i = tap * 2 + kc

                    def mm_fn(mc=mc, tap=tap, dy=dy, dx=dx, kc=kc, i=i):
                        if i == 0:
                            tiles[("psk", mc)] = ps_conv.tile([P, BPC, 6, 6], F32, tag="pss")
                        psk = tiles[("psk", mc)]
                        lhsT = ckw_sb[kc][:, tap * 2 * P + mc * P:tap * 2 * P + (mc + 1) * P]
                        rhs = k_sb[kc][:, :, dy:dy + 6, dx:dx + 6]
                        nc.tensor.matmul(psk[:], lhsT, rhs, start=(i == 0), stop=(i == 17))
                    prev = S.op("PE", 135, gate if i == 0 else [prev], mm_fn)
                    if i == 0:
                        g_psc.chain(prev)

            def relu_fn(mc=mc):
                nc.scalar.activation(krelu_sb[mc][:], tiles[("psk", mc)][:, :, 0:5, 0:5],
                                     AF.Relu, bias=ckb_sb[mc][:])
            r = S.op("ACT", 560, [prev], relu_fn)
            g_psc.use(slot, r)
            krelu_ready.append(r)

    feat_done = {}  # pidx -> (final op, dtype)

    def build_pair(pidx):
        b, cc = pidx // 2, pidx % 2
        lane = XC_CFG[pidx]
        sr = srelu_ready[(b, cc)]
        if lane == "T":
            dslot, gate = g_diag.alloc()
            diag_ops = []
            for tap in range(25):
                def dg_fn(pidx=pidx, cc=cc, b=b, tap=tap):
                    if tap == 0:
                        tiles[("dg", pidx)] = diagp.tile([P, 25, P], F16, tag="diag")
                    nc.vector.tensor_scalar(tiles[("dg", pidx)][:, tap, :], ident_sb[:],
                                            kscalar(cc, b, tap), None, OP.mult)
                deps = (gate + krelu_ready) if tap == 0 else [diag_ops[-1]]
                diag_ops.append(S.op("DVE", 175, deps, dg_fn))
                if tap == 0:
                    g_diag.chain(diag_ops[0])
            xslot, gx = g_psx.alloc()
            prev = None
            for tap in range(25):
                def mm_fn(pidx=pidx, cc=cc, b=b, tap=tap):
                    if tap == 0:
                        tiles[("psa", pidx)] = ps_x.tile([P, 325], F32, tag="psx")
                        tiles[("psb", pidx)] = ps_x.tile([P, 300], F32, tag="psx")
                    dg = tiles[("dg", pidx)]
                    nc.tensor.matmul(tiles[("psa", pidx)][:].rearrange("p (y x) -> p y x", x=25),
                                     dg[:, tap, :], win(cc, b, tap, 0, 13),
                                     start=(tap == 0), stop=(tap == 24))
                    nc.tensor.matmul(tiles[("psb", pidx)][:].rearrange("p (y x) -> p y x", x=25),
                                     dg[:, tap, :], win(cc, b, tap, 13, 12),
                                     start=(tap == 0), stop=(tap == 24))
                deps = [diag_ops[tap]] + sr + (gx if tap == 0 else [prev])
                prev = S.op("PE", 300, deps, mm_fn)
                if tap == 0:
                    g_psx.chain(prev)
            g_diag.use(dslot, prev)

            def cp_fn(pidx=pidx):
                ft = featp.tile([P, 626], F16, tag="feath")
                tiles[pidx] = ft
                nc.gpsimd.memset(ft[:, 625:626], 0.0)
                nc.scalar.activation(ft[:, 0:325], tiles[("psa", pidx)][:], AF.Copy)
                nc.scalar.activation(ft[:, 325:625], tiles[("psb", pidx)][:], AF.Copy)
            fslot, fgate = g_fh.alloc()
            cp = S.op("ACT", 950, [prev] + fgate, cp_fn)
            g_fh.chain(cp)
            g_psx.use(xslot, cp)
            feat_done[pidx] = (cp, F16, g_fh, fslot)
        elif lane == "V":
            prev = None
            for tap in range(25):
                def v_fn(pidx=pidx, cc=cc, b=b, tap=tap):
                    if tap == 0:
                        ft = featp.tile([P, 626], F32R, tag="featf")
                        tiles[pidx] = ft
                        nc.gpsimd.memset(ft[:, 625:626].bitcast(F32), 0.0)
                    fv = tiles[pidx][:, 0:625].rearrange("p (y x) -> p y x", x=25)
                    if tap == 0:
                        nc.vector.tensor_scalar(fv, win(cc, b, 0),
                                                kscalar(cc, b, 0), None, OP.mult)
                    else:
                        nc.vector.scalar_tensor_tensor(fv, win(cc, b, tap),
                                                       kscalar(cc, b, tap),
                                                       fv.bitcast(F32), OP.mult, OP.add)
                if tap == 0:
                    fslot, fgate = g_ff.alloc()
                    deps = sr + krelu_ready + fgate
                else:
                    deps = [prev]
                prev = S.op("DVE", 960, deps, v_fn)
                if tap == 0:
                    g_ff.chain(prev)
            feat_done[pidx] = (prev, F32R, g_ff, fslot)
        else:
            prod_eng = "ACT" if lane == "Gal" else "DVE"
            prod_ops = []
            prod_slots = []
            for tap in range(25):
                pslot, gate = g_prod.alloc()
                prod_slots.append(pslot)

                def pr_fn(pidx=pidx, cc=cc, b=b, tap=tap, lane=lane):
                    pr = prodp.tile([P, 625], F16, tag="prod")
                    tiles[("pr", pidx, tap)] = pr
                    if lane == "Gal":
                        nc.scalar.activation(pr[:].rearrange("p (y x) -> p y x", x=25),
                                             win(cc, b, tap), AF.Copy,
                                             scale=kscalar(cc, b, tap))
                    else:
                        nc.vector.tensor_scalar(pr[:], win(cc, b, tap),
                                                kscalar(cc, b, tap), None, OP.mult)
                prod_ops.append(S.op(prod_eng, 820 if prod_eng == "ACT" else 480,
                                     gate + sr + krelu_ready, pr_fn))
                g_prod.chain(prod_ops[-1])
            if lane == "Md":
                md_fslot = [None]
                items = [(o, ("pr", pidx, t), ("prod", prod_slots[t])) for t, o in enumerate(prod_ops)]
                lvl = 0
                while len(items) > 1:
                    nxt = []
                    for i in range(0, len(items) - 1, 2):
                        a, b_ = items[i], items[i + 1]
                        last = len(items) == 2
                        key = pidx if last else ("tree", pidx, lvl, i)

                        def tr_fn(a=a, b_=b_, key=key, last=last):
                            if last:
                                res = featp.tile([P, 626], F16, tag="feath")
                                tiles[key] = res
                                nc.gpsimd.memset(res[:, 625:626], 0.0)
                                nc.vector.tensor_tensor(res[:, 0:625], tiles[a[1]][:],
                                                        tiles[b_[1]][:], OP.add)
                            else:
                                res = partp.tile([P, 625], F16, tag="part")
                                tiles[key] = res
                                nc.vector.tensor_tensor(res[:], tiles[a[1]][:],
                                                        tiles[b_[1]][:], OP.add)
                        if last:
                            nslot, gate2 = g_fh.alloc()
                            md_fslot[0] = nslot
                        else:
                            nslot, gate2 = g_part.alloc()
                        o = S.op("DVE", 500, [a[0], b_[0]] + gate2, tr_fn)
                        (g_fh if last else g_part).chain(o)
                        for src_ in (a, b_):
                            kind, sl = src_[2]
                            (g_prod if kind == "prod" else g_part).use(sl, o)
                        nxt.append((o, key, ("part", nslot)))
                    if len(items) % 2:
                        nxt.append(items[-1])
                    items = nxt
                    lvl += 1
                feat_done[pidx] = (items[0][0], F16, g_fh, md_fslot[0])
            elif lane == "Gal":
                parts = []
                for i in range(0, 24, 2):
                    ptslot, gate = g_part.alloc()
                    key = ("part", pidx, i)

                    def l1_fn(pidx=pidx, i=i, key=key):
                        pt = partp.tile([P, 625], F16, tag="part")
                        tiles[key] = pt
                        nc.vector.tensor_tensor(pt[:], tiles[("pr", pidx, i)][:],
                                                tiles[("pr", pidx, i + 1)][:], OP.add)
                    o = S.op("DVE", 500, [prod_ops[i], prod_ops[i + 1]] + gate, l1_fn)
                    g_part.chain(o)
                    g_prod.use(prod_slots[i], o)
                    g_prod.use(prod_slots[i + 1], o)
                    parts.append((o, key, ptslot))
                gslot, gacc_gate = g_gacc.alloc()
                prev = None
                for j in range(11):
                    def gs_fn(pidx=pidx, j=j, parts=parts):
                        if j == 0:
                            acc = gaccp.tile([P, 625], F32, tag="gacc")
                            tiles[("acc", pidx)] = acc
                            nc.gpsimd.tensor_tensor(acc[:], tiles[parts[0][1]][:],
                                                    tiles[parts[1][1]][:], OP.add)
                        else:
                            acc = tiles[("acc", pidx)]
                            nc.gpsimd.tensor_tensor(acc[:], tiles[parts[j + 1][1]][:],
                                                    acc[:].bitcast(F32), OP.add)
                    deps = ([parts[0][0], parts[1][0]] + gacc_gate) if j == 0 else [prev, parts[j + 1][0]]
                    prev = S.op("GS", 1600, deps, gs_fn)
                    if j == 0:
                        g_gacc.chain(prev)
                        g_part.use(parts[0][2], prev)
                        g_part.use(parts[1][2], prev)
                    else:
                        g_part.use(parts[j + 1][2], prev)

                def fin_fn(pidx=pidx):
                    ft = featp.tile([P, 626], F32R, tag="featf")
                    tiles[pidx] = ft
                    nc.gpsimd.memset(ft[:, 625:626].bitcast(F32), 0.0)
                    nc.vector.tensor_tensor(ft[:, 0:625], tiles[("pr", pidx, 24)][:],
                                            tiles[("acc", pidx)][:].bitcast(F32), OP.add)
                fslot, fgate = g_ff.alloc()
                fin = S.op("DVE", 800, [prev, prod_ops[24]] + fgate, fin_fn)
                g_ff.chain(fin)
                g_gacc.use(gslot, fin)
                g_prod.use(prod_slots[24], fin)
                feat_done[pidx] = (fin, F32R, g_ff, fslot)
            else:  # Gdf
                gslot, gacc_gate = g_gacc.alloc()
                prev = None
                for j in range(23):
                    def gs_fn(pidx=pidx, j=j):
                        if j == 0:
                            acc = gaccp.tile([P, 625], F32, tag="gacc")
                            tiles[("acc", pidx)] = acc
                            nc.gpsimd.tensor_tensor(acc[:], tiles[("pr", pidx, 0)][:],
                                                    tiles[("pr", pidx, 1)][:], OP.add)
                        else:
                            acc = tiles[("acc", pidx)]
                            nc.gpsimd.tensor_tensor(acc[:], tiles[("pr", pidx, j + 1)][:],
                                                    acc[:].bitcast(F32), OP.add)
                    deps = ([prod_ops[0], prod_ops[1]] + gacc_gate) if j == 0 else [prev, prod_ops[j + 1]]
                    prev = S.op("GS", 1600, deps, gs_fn)
                    if j == 0:
                        g_gacc.chain(prev)
                        g_prod.use(prod_slots[0], prev)
                        g_prod.use(prod_slots[1], prev)
                    else:
                        g_prod.use(prod_slots[j + 1], prev)

                def fin_fn(pidx=pidx):
                    ft = featp.tile([P, 626], F32R, tag="featf")
                    tiles[pidx] = ft
                    nc.gpsimd.memset(ft[:, 625:626].bitcast(F32), 0.0)
                    nc.vector.tensor_tensor(ft[:, 0:625], tiles[("pr", pidx, 24)][:],
                                            tiles[("acc", pidx)][:].bitcast(F32), OP.add)
                fslot, fgate = g_ff.alloc()
                fin = S.op("DVE", 800, [prev, prod_ops[24]] + fgate, fin_fn)
                g_ff.chain(fin)
                g_gacc.use(gslot, fin)
                g_prod.use(prod_slots[24], fin)
                feat_done[pidx] = (fin, F32R, g_ff, fslot)

    def build_head(b):
        f0, dt0, fp0, fs0 = feat_done[b * 2]
        f1, dt1, fp1, fs1 = feat_done[b * 2 + 1]
        fdt = {0: dt0, 1: dt1}
        fpool = {0: (fp0, fs0), 1: (fp1, fs1)}
        xslot, xgate = g_xr.alloc()

        def xr_alloc_fn(b=b):
            for m2 in range(KC):
                t = xrp.tile([P, 626], F32R, tag="xr", name=f"xr{b}_{m2}")
                tiles[("xr", b, m2)] = t
                nc.gpsimd.memset(t[:, 625:626].bitcast(F32), 0.0)
        xr_op = S.op("GS", 150, xgate, xr_alloc_fn)
        g_xr.chain(xr_op)
        xr_relu = []
        for mc in range(KC):
            for o0, n, nv in ((0, 320, 320), (320, 306, 305)):
                hslot, gate = g_pshd.alloc()
                prev = None
                for kc in range(KC):
                    def mm_fn(b=b, mc=mc, o0=o0, n=n, kc=kc):
                        if kc == 0:
                            tiles[("ph", b, mc, o0)] = ps_hd.tile([P, n], F32, tag="pshd")
                        ft = tiles[b * 2 + kc]
                        lhsT = (h1w16_sb if fdt[kc] == F16 else h1w_sb)[kc][:, mc * P:(mc + 1) * P]
                        nc.tensor.matmul(tiles[("ph", b, mc, o0)][:], lhsT, ft[:, o0:o0 + n],
                                         start=(kc == 0), stop=(kc == 1))
                    deps = ([f0, f1] + gate) if kc == 0 else [prev]
                    prev = S.op("PE", 145, deps, mm_fn)
                    if kc == 0:
                        g_pshd.chain(prev)
                    fpool[kc][0].use(fpool[kc][1], prev)

                def relu_fn(b=b, mc=mc, o0=o0, nv=nv):
                    nc.scalar.activation(tiles[("xr", b, mc)][:, o0:o0 + nv],
                                         tiles[("ph", b, mc, o0)][:, 0:nv],
                                         AF.Relu, bias=h1b_sb[mc][:])
                r = S.op("ACT", 560, [prev, xr_op], relu_fn)
                g_pshd.use(hslot, r)
                xr_relu.append(r)
        for o0, n, nv in ((0, 320, 320), (320, 306, 305)):
            hslot, gate = g_pshd.alloc()
            prev = None
            for kc in range(KC):
                def mm_fn(b=b, o0=o0, n=n, kc=kc):
                    if kc == 0:
                        tiles[("po", b, o0)] = ps_hd.tile([OUT, n], F32, tag="pshd", name=f"po{b}_{o0}")
                    nc.tensor.matmul(tiles[("po", b, o0)][:], h2w_sb[kc][:],
                                     tiles[("xr", b, kc)][:, o0:o0 + n],
                                     start=(kc == 0), stop=(kc == 1))
                deps = (xr_relu + gate) if kc == 0 else [prev]
                prev = S.op("PE", 140, deps, mm_fn)
                if kc == 0:
                    g_pshd.chain(prev)
                g_xr.use(xslot, prev)

            def out_fn(b=b, o0=o0, nv=nv):
                nc.scalar.activation(out_sb[:, b * 625 + o0:b * 625 + o0 + nv],
                                     tiles[("po", b, o0)][:, 0:nv],
                                     AF.Identity, bias=h2b_sb[:])
            oc = S.op("ACT", 560, [prev], out_fn)
            g_pshd.use(hslot, oc)

    build_conv_kernel()
    for b in range(BPC):
        build_conv(b)
    for pidx in range(2 * BPC):
        build_pair(pidx)
    for b in range(BPC):
        build_head(b)

    for o in S.order():
        o.fn()

    nc.sync.dma_start(d["out"].ap(), out_sb[:])


def _build_program():
    if "nc" in _prog_cache:
        return _prog_cache["nc"]
    nc = bacc.Bacc("TRN2", target_bir_lowering=False, debug=False,
                   num_devices=NCORES)
    d = {}
    d["s_in"] = nc.dram_tensor("s_in", [KC, P, BPC, 31, 32], BF16, kind="ExternalInput")
    d["k_in"] = nc.dram_tensor("k_in", [KC, P, BPC, 9, 9], BF16, kind="ExternalInput")
    d["csw"] = nc.dram_tensor("csw", [KC, P, 9, 2, P], BF16, kind="ExternalInput")
    d["ckw"] = nc.dram_tensor("ckw", [KC, P, 9, 2, P], BF16, kind="ExternalInput")
    d["cs_bias"] = nc.dram_tensor("cs_bias", [KC, P, 1], F32, kind="ExternalInput")
    d["ck_bias"] = nc.dram_tensor("ck_bias", [KC, P, 1], F32, kind="ExternalInput")
    d["h1w"] = nc.dram_tensor("h1w", [KC, P, 2, P], F32R, kind="ExternalInput")
    d["h1w16"] = nc.dram_tensor("h1w16", [KC, P, 2, P], F16, kind="ExternalInput")
    d["h1_bias"] = nc.dram_tensor("h1_bias", [KC, P, 1], F32, kind="ExternalInput")
    d["h2w"] = nc.dram_tensor("h2w", [KC, P, OUT], F32R, kind="ExternalInput")
    d["h2_bias"] = nc.dram_tensor("h2_bias", [OUT, 1], F32, kind="ExternalInput")
    d["ident"] = nc.dram_tensor("ident", [P, P], F16, kind="ExternalInput")
    d["out"] = nc.dram_tensor("out", [OUT, BPC * 625], F32, kind="ExternalOutput")

    with tile.TileContext(nc) as tc:
        with ExitStack() as ctx:
            _emit(nc, tc, ctx, d)
    nc.compile()
    _prog_cache["nc"] = nc
    return nc


def kernel(**inputs):
    global LAST_RESULTS
    f32 = lambda x: np.ascontiguousarray(np.asarray(x), dtype=np.float32)
    kern, search = f32(inputs["kernel"]), f32(inputs["search"])

    # fold BN into conv weights / biases
    cks = f32(inputs["ck_g"]) / np.sqrt(f32(inputs["ck_v"]) + EPS)
    ckw_f = f32(inputs["ck_w"]) * cks[:, None, None, None]
    ckb = f32(inputs["ck_b"]) - f32(inputs["ck_m"]) * cks
    css = f32(inputs["cs_g"]) / np.sqrt(f32(inputs["cs_v"]) + EPS)
    csw_f = f32(inputs["cs_w"]) * css[:, None, None, None]
    csb = f32(inputs["cs_b"]) - f32(inputs["cs_m"]) * css
    h1s = f32(inputs["h_g"]) / np.sqrt(f32(inputs["h_v"]) + EPS)
    h1w_f = f32(inputs["h1_w"]) * h1s[:, None]
    h1b = f32(inputs["h_b"]) - f32(inputs["h_m"]) * h1s

    h1w_t = np.ascontiguousarray(h1w_f.transpose(1, 0).reshape(KC, P, 2, P))
    shared = {
        "csw": np.ascontiguousarray(
            csw_f.transpose(1, 2, 3, 0).reshape(KC, P, 9, 2, P)).astype(ml_dtypes.bfloat16),
        "ckw": np.ascontiguousarray(
            ckw_f.transpose(1, 2, 3, 0).reshape(KC, P, 9, 2, P)).astype(ml_dtypes.bfloat16),
        "cs_bias": csb.reshape(KC, P, 1),
        "ck_bias": ckb.reshape(KC, P, 1),
        "h1w": h1w_t,
        "h1w16": h1w_t.astype(np.float16),
        "h1_bias": h1b.reshape(KC, P, 1),
        "h2w": np.ascontiguousarray(f32(inputs["h2_w"]).transpose(1, 0).reshape(KC, P, OUT)),
        "h2_bias": f32(inputs["h2_b"]).reshape(OUT, 1),
        "ident": np.eye(P, dtype=np.float16),
    }
    in_maps = []
    for i in range(NCORES):
        sl = slice(i * BPC, (i + 1) * BPC)
        m = dict(shared)
        s_pad = np.zeros((KC, P, BPC, 31, 32), ml_dtypes.bfloat16)
        s_pad[..., :31] = search[sl].transpose(1, 0, 2, 3).reshape(KC, P, BPC, 31, 31)
        m["s_in"] = s_pad
        k_pad = np.zeros((KC, P, BPC, 9, 9), ml_dtypes.bfloat16)
        k_pad[..., :7, :7] = kern[sl].transpose(1, 0, 2, 3).reshape(KC, P, BPC, 7, 7)
        m["k_in"] = k_pad
        in_maps.append(m)

    nc = _build_program()
    res = run_bass_kernel_spmd(nc, in_maps, core_ids=list(range(NCORES)))
    LAST_RESULTS = res
    out = np.empty((B, OUT, 25, 25), dtype=np.float32)
    for i in range(NCORES):
        o = res.results[i]["out"].reshape(OUT, BPC, 25, 25)
        out[i * BPC:(i + 1) * BPC] = o.transpose(1, 0, 2, 3)
    return out


# revision 13
# speedup vs baseline: 1.1208x; 1.0806x over previous
"""Trainium2 Bass kernel for DepthwiseXCorr (SiamRPN-style) model.

Pipeline (per sample): conv3x3+BN+ReLU on kernel & search branches,
depthwise cross-correlation, 1x1 conv + BN + ReLU head, 1x1 conv + bias.

Sharding: data-parallel over batch across 8 NeuronCores (8 samples each),
weights replicated.  BN is folded into conv weights on the host.

Layout on device: channels on SBUF partitions (2 chunks of 128), spatial x
batch on the free dimension.  Convolutions run as 9 shifted matmuls — the
search branch in bf16 (fast FWL weight loads), the kernel branch in fp32r.
The depthwise xcorr is split per (sample, channel-chunk) pair between the
TensorEngine (accumulating diag(k_tap) @ shifted-window matmuls in PSUM,
diagonals built by ScalarE from a bf16 identity) and the VectorEngine
(scalar_tensor_tensor chains over one contiguous span per tap), with the
pair assignment tuned so both engines finish around the same time and the
tail drains on the TensorEngine.  Emission is interleaved per sample so
every engine's instruction stream pipelines conv -> xcorr -> head.
"""

import sys

if "/opt/trn_rl_repo" not in sys.path:
    sys.path.insert(0, "/opt/trn_rl_repo")

from contextlib import ExitStack

import ml_dtypes
import numpy as np

import concourse.bass as bass
import concourse.tile as tile
from concourse import bacc, mybir
from concourse.bass_utils import run_bass_kernel_spmd

EPS = 1e-5
NCORES = 8
B, C, HID, OUT = 64, 256, 256, 10
BPC = B // NCORES  # samples per core
P = 128
KC = C // P  # channel chunks (2)
F32 = mybir.dt.float32
F32R = mybir.dt.float32r
BF16 = mybir.dt.bfloat16
AF = mybir.ActivationFunctionType
OP = mybir.AluOpType

# xcorr engine per (b, cc) pair, index p = b*2 + cc
# 't' = TensorE diag-matmul, 'v' = VectorE, 'g' = GpSimd
XC_ENGINE = ["v", "t", "v", "t", "v", "t", "v", "t",
             "v", "t", "v", "v", "t", "t", "t", "t"]

LAST_RESULTS = None  # BassKernelResults of the most recent run (for profiling)

_prog_cache = {}


def _emit(nc, tc, ctx, d):
    """Emit the per-core program.  d maps dram tensor name -> handle."""
    wp = ctx.enter_context(tc.tile_pool(name="weights", bufs=1))
    srp = ctx.enter_context(tc.tile_pool(name="srelu", bufs=1))
    krp = ctx.enter_context(tc.tile_pool(name="krelu", bufs=1))
    kp = ctx.enter_context(tc.tile_pool(name="kern", bufs=1))
    sp = ctx.enter_context(tc.tile_pool(name="search", bufs=8))
    featp = ctx.enter_context(tc.tile_pool(name="feat", bufs=8))
    diagp = ctx.enter_context(tc.tile_pool(name="diag", bufs=6))
    xrp = ctx.enter_context(tc.tile_pool(name="xrelu", bufs=6))
    outp = ctx.enter_context(tc.tile_pool(name="outs", bufs=1))
    ps_conv = ctx.enter_context(tc.tile_pool(name="ps_conv", bufs=3, space="PSUM"))
    ps_x = ctx.enter_context(tc.tile_pool(name="ps_x", bufs=3, space="PSUM"))
    ps_hd = ctx.enter_context(tc.tile_pool(name="ps_hd", bufs=2, space="PSUM"))

    # ---- weights / constants into SBUF ----
    s0_sb = []
    for kc in range(KC):
        t = sp.tile([P, 31, 32], BF16, tag="sin", name=f"sin{kc}_0")
        nc.sync.dma_start(t[:], d["s_in"].ap()[kc, :, 0])
        s0_sb.append(t)
    csw_sb, ckw_sb, h1w_sb, h2w_sb = [], [], [], []
    csb_sb, ckb_sb, h1b_sb = [], [], []
    for kc in range(KC):
        t = wp.tile([P, 9 * 2 * P], BF16, tag=f"csw{kc}")
        nc.sync.dma_start(t[:], d["csw"].ap()[kc])
        csw_sb.append(t)
    k_sb = []
    for kc in range(KC):
        t = kp.tile([P, BPC, 9, 9], BF16, tag=f"kin{kc}")
        nc.sync.dma_start(t[:], d["k_in"].ap()[kc])
        k_sb.append(t)
    for kc in range(KC):
        t = wp.tile([P, 9 * 2 * P], BF16, tag=f"ckw{kc}")
        nc.sync.dma_start(t[:], d["ckw"].ap()[kc])
        ckw_sb.append(t)
    for kc in range(KC):
        t = wp.tile([P, 2 * P], F32R, tag=f"h1w{kc}")
        nc.sync.dma_start(t[:], d["h1w"].ap()[kc])
        h1w_sb.append(t)
        t = wp.tile([P, OUT], F32R, tag=f"h2w{kc}")
        nc.sync.dma_start(t[:], d["h2w"].ap()[kc])
        h2w_sb.append(t)
    for mc in range(KC):
        t = wp.tile([P, 1], F32, tag=f"csb{mc}")
        nc.sync.dma_start(t[:], d["cs_bias"].ap()[mc])
        csb_sb.append(t)
        t = wp.tile([P, 1], F32, tag=f"ckb{mc}")
        nc.sync.dma_start(t[:], d["ck_bias"].ap()[mc])
        ckb_sb.append(t)
        t = wp.tile([P, 1], F32, tag=f"h1b{mc}")
        nc.sync.dma_start(t[:], d["h1_bias"].ap()[mc])
        h1b_sb.append(t)
    h2b_sb = wp.tile([OUT, 1], F32, tag="h2b")
    nc.sync.dma_start(h2b_sb[:], d["h2_bias"].ap())
    ident_sb = wp.tile([P, P], BF16, tag="ident")
    nc.sync.dma_start(ident_sb[:], d["ident"].ap())

    # ---- inputs + convolutions (rotating per-sample search tiles) ----
    krelu_sb = [krp.tile([P, BPC * 25], F32, tag=f"krelu{mc}", name=f"krelu{mc}") for mc in range(KC)]
    srelu_sb = [srp.tile([P, BPC, 29, 30], BF16, tag=f"srelu{mc}", name=f"srelu{mc}") for mc in range(KC)]
    for mc in range(KC):
        nc.vector.memset(srelu_sb[mc][:, :, :, 29:30], 0.0)

    # ---- depthwise xcorr + head, pipelined per sample ----
    out_sb = outp.tile([OUT, BPC * 625], F32, tag="osb")

    def kscalar(cc, b, tap):
        return krelu_sb[cc][:, b * 25 + tap:b * 25 + tap + 1]

    def win(cc, b, tap, r0=0, nr=25, w=25):
        dy, dx = tap // 5, tap % 5
        return srelu_sb[cc][:, b, dy + r0:dy + r0 + nr, dx:dx + w]

    def win746(cc, b, tap):
        # contiguous 746-elem span of the (dy,dx)-shifted window
        dy, dx = tap // 5, tap % 5
        flat = srelu_sb[cc][:].rearrange("p b y x -> p (b y x)")
        off = b * 870 + dy * 30 + dx
        return flat[:, off:off + 746]

    def ftwin(ft, r0, nr):
        # [nr, 26] row window of the 30-stride feat tile
        return ft[:].rearrange("p (y x) -> p y x", x=30)[:, r0:r0 + nr, 0:26]

    def ftwin25(ft, r0, nr):
        return ft[:].rearrange("p (y x) -> p y x", x=30)[:, r0:r0 + nr, 0:25]

    for b in range(BPC):
        # conv_search(b): s_relu[mc][b] = [29, 30] (col 29 = padding junk)
        if b == 0:
            s_sb = s0_sb
        else:
            s_sb = []
            for kc in range(KC):
                t = sp.tile([P, 31, 32], BF16, tag="sin", name=f"sin{kc}_{b}")
                nc.sync.dma_start(t[:], d["s_in"].ap()[kc, :, b])
                s_sb.append(t)
        for mc in range(KC):
            for y0, nr in ((0, 15), (15, 14)):
                pss = ps_conv.tile([P, nr, 29], F32, tag="pss")
                i = 0
                for tap in range(9):
                    dy, dx = tap // 3, tap % 3
                    for kc in range(KC):
                        lhsT = csw_sb[kc][:, tap * 2 * P + mc * P:tap * 2 * P + (mc + 1) * P]
                        rhs = s_sb[kc][:, y0 + dy:y0 + dy + nr, dx:dx + 29]
                        nc.tensor.matmul(pss[:], lhsT, rhs,
                                         start=(i == 0), stop=(i == 17))
                        i += 1
                nc.scalar.activation(srelu_sb[mc][:, b, y0:y0 + nr, 0:29], pss[:],
                                     AF.Relu, bias=csb_sb[mc][:])

        if b == 0:
            # conv_kernel (3x3, BN+ReLU folded): k_relu[mc] = [128, b*25+tap]
            for mc in range(KC):
                psk = ps_conv.tile([P, BPC, 6, 6], F32, tag="pss")
                i = 0
                for tap in range(9):
                    dy, dx = tap // 3, tap % 3
                    for kc in range(KC):
                        lhsT = ckw_sb[kc][:, tap * 2 * P + mc * P:tap * 2 * P + (mc + 1) * P]
                        rhs = k_sb[kc][:, :, dy:dy + 6, dx:dx + 6]
                        nc.tensor.matmul(psk[:], lhsT, rhs, start=(i == 0), stop=(i == 17))
                        i += 1
                nc.scalar.activation(krelu_sb[mc][:], psk[:, :, 0:5, 0:5], AF.Relu,
                                     bias=ckb_sb[mc][:])

        # xcorr(b-1 pipelining handled by Tile deps)
        feat = []
        for cc in range(KC):
            eng = XC_ENGINE[b * 2 + cc]
            # ft holds the 25x25 xcorr output on a 30-element row stride
            # (cols 25..29 junk) so DVE ops can run one contiguous span.
            ft = featp.tile([P, 750], F32R, tag="feat")
            if eng == "v":
                e = nc.vector
                e.tensor_scalar(ft[:, 0:746], win746(cc, b, 0),
                                kscalar(cc, b, 0), None, OP.mult)
                for tap in range(1, 25):
                    e.scalar_tensor_tensor(ft[:, 0:746], win746(cc, b, tap),
                                           kscalar(cc, b, tap),
                                           ft[:, 0:746].bitcast(F32), OP.mult, OP.add)
            else:  # TensorE: accumulate diag(k_tap) @ shifted windows in PSUM
                pe_taps = 13 if eng == "s" else 25
                ps_a = ps_x.tile([P, 13, 25], F32, tag="psx")
                ps_b = ps_x.tile([P, 12, 25], F32, tag="psx")
                for tap in range(pe_taps):
                    dg = diagp.tile([P, P], BF16, tag="diag")
                    nc.scalar.activation(dg[:], ident_sb[:], AF.Copy,
                                         scale=kscalar(cc, b, tap))
                    nc.tensor.matmul(ps_a[:], dg[:],
                                     win(cc, b, tap, 0, 13, 25),
                                     start=(tap == 0), stop=(tap == pe_taps - 1))
                    nc.tensor.matmul(ps_b[:], dg[:],
                                     win(cc, b, tap, 13, 12, 25),
                                     start=(tap == 0), stop=(tap == pe_taps - 1))
                nc.scalar.activation(ftwin25(ft, 0, 13), ps_a[:], AF.Copy)
                nc.scalar.activation(ftwin25(ft, 13, 12), ps_b[:], AF.Copy)
                if eng == "s":
                    # junk cols of ft (25..29 per row) never ACT-copied; zero
                    # them so the 746-span DVE chain stays NaN-free
                    jnk = ft[:].rearrange("p (y x) -> p y x", x=30)[:, :, 25:30]
                    nc.vector.memset(jnk.bitcast(F32), 0.0)
                    for tap in range(pe_taps, 25):
                        nc.vector.scalar_tensor_tensor(
                            ft[:, 0:746], win746(cc, b, tap), kscalar(cc, b, tap),
                            ft[:, 0:746].bitcast(F32), OP.mult, OP.add)
            feat.append(ft)

        # head1: 1x1 conv + BN + ReLU (row-aligned splits over the 25x26 window)
        xr = []
        for mc in range(KC):
            t = xrp.tile([P, 626], F32R, tag="xr", name=f"xr{b}_{mc}")
            nc.vector.memset(t[:, 625:626].bitcast(F32), 0.0)
            xr.append(t)
        for mc in range(KC):
            for r0, nr in ((0, 13), (13, 12)):
                ph = ps_hd.tile([P, nr, 26], F32, tag="pshd")
                for kc in range(KC):
                    lhsT = h1w_sb[kc][:, mc * P:(mc + 1) * P]
                    nc.tensor.matmul(ph[:], lhsT, ftwin(feat[kc], r0, nr),
                                     start=(kc == 0), stop=(kc == 1))
                nc.scalar.activation(xr[mc][:, r0 * 25:(r0 + nr) * 25],
                                     ph[:, :, 0:25], AF.Relu, bias=h1b_sb[mc][:])

        # head2: 1x1 conv + bias
        for o0, n, nv in ((0, 320, 320), (320, 306, 305)):
            po = ps_hd.tile([OUT, n], F32, tag="pshd")
            for kc in range(KC):
                nc.tensor.matmul(po[:], h2w_sb[kc][:],
                                 xr[kc][:, o0:o0 + n],
                                 start=(kc == 0), stop=(kc == 1))
            nc.scalar.activation(out_sb[:, b * 625 + o0:b * 625 + o0 + nv], po[:, 0:nv],
                                 AF.Identity, bias=h2b_sb[:])

    nc.sync.dma_start(d["out"].ap(), out_sb[:])


def _build_program():
    if "nc" in _prog_cache:
        return _prog_cache["nc"]
    nc = bacc.Bacc("TRN2", target_bir_lowering=False, debug=False,
                   num_devices=NCORES)
    d = {}
    d["s_in"] = nc.dram_tensor("s_in", [KC, P, BPC, 31, 32], BF16, kind="ExternalInput")
    d["k_in"] = nc.dram_tensor("k_in", [KC, P, BPC, 9, 9], BF16, kind="ExternalInput")
    d["csw"] = nc.dram_tensor("csw", [KC, P, 9, 2, P], BF16, kind="ExternalInput")
    d["ckw"] = nc.dram_tensor("ckw", [KC, P, 9, 2, P], BF16, kind="ExternalInput")
    d["cs_bias"] = nc.dram_tensor("cs_bias", [KC, P, 1], F32, kind="ExternalInput")
    d["ck_bias"] = nc.dram_tensor("ck_bias", [KC, P, 1], F32, kind="ExternalInput")
    d["h1w"] = nc.dram_tensor("h1w", [KC, P, 2, P], F32R, kind="ExternalInput")
    d["h1_bias"] = nc.dram_tensor("h1_bias", [KC, P, 1], F32, kind="ExternalInput")
    d["h2w"] = nc.dram_tensor("h2w", [KC, P, OUT], F32R, kind="ExternalInput")
    d["h2_bias"] = nc.dram_tensor("h2_bias", [OUT, 1], F32, kind="ExternalInput")
    d["ident"] = nc.dram_tensor("ident", [P, P], BF16, kind="ExternalInput")
    d["out"] = nc.dram_tensor("out", [OUT, BPC * 625], F32, kind="ExternalOutput")

    with tile.TileContext(nc) as tc:
        with ExitStack() as ctx:
            _emit(nc, tc, ctx, d)
    nc.compile()
    _prog_cache["nc"] = nc
    return nc


def kernel(**inputs):
    global LAST_RESULTS
    f32 = lambda x: np.ascontiguousarray(np.asarray(x), dtype=np.float32)
    kern, search = f32(inputs["kernel"]), f32(inputs["search"])

    # fold BN into conv weights / biases
    cks = f32(inputs["ck_g"]) / np.sqrt(f32(inputs["ck_v"]) + EPS)
    ckw_f = f32(inputs["ck_w"]) * cks[:, None, None, None]
    ckb = f32(inputs["ck_b"]) - f32(inputs["ck_m"]) * cks
    css = f32(inputs["cs_g"]) / np.sqrt(f32(inputs["cs_v"]) + EPS)
    csw_f = f32(inputs["cs_w"]) * css[:, None, None, None]
    csb = f32(inputs["cs_b"]) - f32(inputs["cs_m"]) * css
    h1s = f32(inputs["h_g"]) / np.sqrt(f32(inputs["h_v"]) + EPS)
    h1w_f = f32(inputs["h1_w"]) * h1s[:, None]
    h1b = f32(inputs["h_b"]) - f32(inputs["h_m"]) * h1s

    shared = {
        "csw": np.ascontiguousarray(
            csw_f.transpose(1, 2, 3, 0).reshape(KC, P, 9, 2, P)).astype(ml_dtypes.bfloat16),
        "ckw": np.ascontiguousarray(
            ckw_f.transpose(1, 2, 3, 0).reshape(KC, P, 9, 2, P)).astype(ml_dtypes.bfloat16),
        "cs_bias": csb.reshape(KC, P, 1),
        "ck_bias": ckb.reshape(KC, P, 1),
        "h1w": np.ascontiguousarray(h1w_f.transpose(1, 0).reshape(KC, P, 2, P)),
        "h1_bias": h1b.reshape(KC, P, 1),
        "h2w": np.ascontiguousarray(f32(inputs["h2_w"]).transpose(1, 0).reshape(KC, P, OUT)),
        "h2_bias": f32(inputs["h2_b"]).reshape(OUT, 1),
        "ident": np.eye(P, dtype=ml_dtypes.bfloat16),
    }
    in_maps = []
    for i in range(NCORES):
        sl = slice(i * BPC, (i + 1) * BPC)
        m = dict(shared)
        s_pad = np.zeros((KC, P, BPC, 31, 32), ml_dtypes.bfloat16)
        s_pad[..., :31] = search[sl].transpose(1, 0, 2, 3).reshape(KC, P, BPC, 31, 31)
        m["s_in"] = s_pad
        k_pad = np.zeros((KC, P, BPC, 9, 9), ml_dtypes.bfloat16)
        k_pad[..., :7, :7] = kern[sl].transpose(1, 0, 2, 3).reshape(KC, P, BPC, 7, 7)
        m["k_in"] = k_pad
        in_maps.append(m)

    nc = _build_program()
    res = run_bass_kernel_spmd(nc, in_maps, core_ids=list(range(NCORES)))
    LAST_RESULTS = res
    out = np.empty((B, OUT, 25, 25), dtype=np.float32)
    for i in range(NCORES):
        o = res.results[i]["out"].reshape(OUT, BPC, 25, 25)
        out[i * BPC:(i + 1) * BPC] = o.transpose(1, 0, 2, 3)
    return out

